# revision 8
# baseline (speedup 1.0000x reference)
"""AttentiveRNN Trainium2 kernel, v2.

Same math as the baseline kernel (8-way parallel warm-up scan + folded
attention), restructured to cut instruction count and device time:

- x DMA'd in 4 big slabs; h = relu(W_in x + b_in) hoisted into one batched
  pass (32 matmuls) before the scan instead of per-step matmuls.
- Scan critical path: per step only the wcc matmul sits between relu(j-1)
  and relu(j); the wch@h matmul is issued first (PSUM accumulate order is
  commutative) so PE does it while waiting.
- G = M_hat @ [C;1] batched over all 64 batch elements in one 33-matmul
  pass (replaces 64 per-b matmuls + 64 DVE copies).
- Attention processed in pairs: both score chunks of both elements of a
  pair land in one 2-bank PSUM tile, giving ONE exp over [128,2,384] and
  two paired affine_select masks per pair.
- All small weights packed into a single input tensor (2 inputs + 2
  outputs total vs 10+3).

Host postprocess (unchanged): softmax normalization, + b_act, and the two
diagonal terms the device skips ((t=127,s=128) and (t=255,s=256)).
"""

import sys
from contextlib import ExitStack

sys.path.insert(0, "/opt/trn_rl_repo")

import numpy as np

import concourse.bacc as bacc
import concourse.bass as bass
import concourse.tile as tile
from concourse import mybir

T, B, D, H, K, A = 256, 512, 128, 50, 5, 4
KP = 6  # K padded even so caps/acps matmuls stay on the f32r fast path
N_CORES = 8
BC = B // N_CORES  # 64 batch elements per core
S = T + 1  # context count
F32 = mybir.dt.float32
F32R = mybir.dt.float32r
BF16 = mybir.dt.bfloat16
AF = mybir.ActivationFunctionType

NCg = 8  # parallel scan chunks
W_WARM = 8  # warmup steps; relu recurrence contracts ~0.28x/step -> ~4e-5 rel
L1 = (T - W_WARM) // NCg  # 31
S_CH = L1 + W_WARM  # 39 scan steps

# wpack column layout (per-core packed weights, [128, CW] f32 bits)
C_WP1 = 0          # [0:128, 0:50]   W_in^T
C_WCC = 50         # [0:50, 50:100]  Wctx[:, :H]^T
C_WCH = 100        # [0:50, 100:150] Wctx[:, H:]^T
C_MH = 150         # [0:51, 150:201] M_hat^T
C_WAE = 202        # [0:51, 202:208] [W_act^T | e_A | 0]
C_BIN = 208        # [0:50, 208]     b_in
C_BCTX = 209       # [0:50, 209]     b_ctx
C_C0 = 210         # [0:51, 210:722] c0 broadcast to 8*64 (+ones row 50)
CW = 722

_CACHE = {}


def _build_nc(reps=1, stage=4):
    # stage: 1=h only, 2=+scan, 3=+G, 4=full
    # 5=full minus affine_selects, 6=full minus caps/acps, 7=exp->DVE copy
    nc = bacc.Bacc("TRN2", target_bir_lowering=False, debug=False)

    # x columns pre-permuted on host: position j*8+i holds t=i*L1+j (j<L1),
    # tail positions 8*L1.. hold t=7*L1+j (j>=L1).
    xT = nc.dram_tensor("xT", [D, T, BC], F32R, kind="ExternalInput")
    wpack = nc.dram_tensor("wpack", [D, CW], F32R, kind="ExternalInput")

    acts_raw = nc.dram_tensor("acts_raw", [128, 2, BC, KP], F32, kind="ExternalOutput")
    c_edge = nc.dram_tensor("c_edge", [H, 2, BC], BF16, kind="ExternalOutput")

    with tile.TileContext(nc) as tc:
        rep_stack = ExitStack()
        if reps > 1:
            rep_stack.enter_context(
                tc.For_i(0, reps, 1, hint_engines=(mybir.EngineType.PE,))
            )
        with tc.tile_pool(name="persist", bufs=1) as persist:
            wsb = persist.tile([D, CW], F32R, tag="wsb")
            nc.sync.dma_start(wsb, wpack[:])
            wp1 = wsb[0:D, C_WP1 : C_WP1 + H]
            wcc = wsb[0:H, C_WCC : C_WCC + H]
            wch = wsb[0:H, C_WCH : C_WCH + H]
            mh = wsb[0 : H + 1, C_MH : C_MH + H + 1]
            wae = wsb[0 : H + 1, C_WAE : C_WAE + KP]
            bin_ = wsb[0:H, C_BIN : C_BIN + 1].bitcast(F32)
            bctx = wsb[0:H, C_BCTX : C_BCTX + 1].bitcast(F32)
            c0rep = wsb[0 : H + 1, C_C0 : C_C0 + NCg * BC]

            mh_bf = persist.tile([H + 1, H + 1], BF16, tag="mh_bf")
            nc.vector.tensor_copy(mh_bf, mh.bitcast(F32))
            wae_bf = persist.tile([H + 1, KP], BF16, tag="wae_bf")
            nc.vector.tensor_copy(wae_bf, wae.bitcast(F32))

            # CAT rows 0-49: context c_{s-1} per block s; row 50: ones.
            # The ones row rides along with every commit: CST/c0rep carry a
            # constant-1.0 row 50 that each 51-row commit copies into CAT.
            CAT = persist.tile([H + 1, S + 1, BC], BF16)  # +1 pad block
            CST = persist.tile([H + 1, 3, NCg, BC], F32R)  # scan state (3-slot)
            ACTS = persist.tile([128, 2, BC, KP], F32)

            nc.gpsimd.tensor_copy(CAT[0 : H + 1, 0:1, :], c0rep[:, 0:BC])
            nc.gpsimd.tensor_copy(CAT[0 : H + 1, S : S + 1, :], c0rep[:, 0:BC])
            for sl3 in range(3):
                nc.gpsimd.tensor_copy(CST[:, sl3, :, :], c0rep[:])

            # ---- x DMA (8 slabs, double-buffered) interleaved with the
            # hoisted h pass AND the scan, so the scan starts immediately
            # (PE queue is in-order; h matmul j feeds scan step j).
            NSLAB = 8
            TB = T // NSLAB  # 32 blocks per slab
            HPS = TB // NCg  # 4 h-matmuls per slab
            with tc.tile_pool(name="scanp", bufs=1) as scanp:
                HA = scanp.tile([H, T, BC], F32R)  # h = relu(W_in x + b)
                with (
                    tc.tile_pool(name="xp", bufs=2) as xp,
                    tc.tile_pool(name="psH", bufs=2, space=bass.MemorySpace.PSUM) as psH,
                    tc.tile_pool(name="psC", bufs=2, space=bass.MemorySpace.PSUM) as psCp,
                ):
                    xbs = {}
                    xb0 = xp.tile([D, TB, BC], F32R, tag="xb")
                    xbs[0] = xb0
                    nc.sync.dma_start(xbs[0], xT[:, 0:TB, :])

                    def h_pair(k):
                        # h-matmuls k and k+1 share one PSUM tile + relu pair
                        sl, kk = divmod(k, HPS)
                        if kk == 0 and sl + 1 < NSLAB:
                            xbn = xp.tile([D, TB, BC], F32R, tag="xb")
                            xbs[sl + 1] = xbn
                            nc.sync.dma_start(
                                xbn, xT[:, (sl + 1) * TB : (sl + 2) * TB, :]
                            )
                        ph = psH.tile([H, 2 * NCg, BC], F32, tag="ph")
                        nc.tensor.matmul(
                            ph[:, 0:NCg, :],
                            wp1,
                            xbs[sl][:, kk * NCg : (kk + 1) * NCg, :],
                            skip_group_check=True,
                        )
                        nc.tensor.matmul(
                            ph[:, NCg : 2 * NCg, :],
                            wp1,
                            xbs[sl][:, (kk + 1) * NCg : (kk + 2) * NCg, :],
                            skip_group_check=True,
                        )
                        hs = NCg  # bank boundary of ph
                        dst = HA[:, k * NCg : (k + 2) * NCg, :]
                        nc.scalar.activation(
                            dst[:, 0:hs, :], ph[:, 0:hs, :], AF.Relu, bias=bin_
                        )
                        nc.vector.tensor_scalar(
                            dst[:, hs:, :],
                            ph[:, hs:, :],
                            bin_[:],
                            0.0,
                            op0=mybir.AluOpType.add,
                            op1=mybir.AluOpType.max,
                        )

                    def scan_step(j):
                        # pc: [H, 2, 512] = one PSUM bank per half, so the
                        # ACT and DVE relu halves run truly parallel (same-
                        # bank pairs are serialized by the bank tracker).
                        # Each wcc half reads/writes only its own chunk
                        # range, so the two chains decouple step-to-step.
                        pc0 = psCp.tile([H, 256], F32, tag="pc0")
                        pc1 = psCp.tile([H, 256], F32, tag="pc1")
                        pcs = (pc0, pc1)
                        hf = NCg // 2
                        for q in range(2):
                            pq = pcs[q][:, 0:256]
                            # wch@h first: no dep on relu(j-1).
                            if j < L1:
                                nc.tensor.matmul(
                                    pq,
                                    wch,
                                    HA[:, j * NCg + q * hf : j * NCg + (q + 1) * hf, :],
                                    start=True,
                                    stop=False,
                                    skip_group_check=True,
                                )
                            else:
                                jp = j - L1
                                if q == 0:
                                    nc.tensor.matmul(
                                        pq,
                                        wch,
                                        HA[:, jp * NCg + 1 : jp * NCg + 1 + hf, :],
                                        start=True,
                                        stop=False,
                                        skip_group_check=True,
                                    )
                                else:
                                    nc.tensor.matmul(
                                        pc1[:, 0:192],
                                        wch,
                                        HA[:, jp * NCg + 5 : jp * NCg + NCg, :],
                                        start=True,
                                        stop=False,
                                        skip_group_check=True,
                                    )
                                    p = NCg * L1 + jp
                                    nc.tensor.matmul(
                                        pc1[:, 192:256],
                                        wch,
                                        HA[:, p : p + 1, :],
                                        start=False,
                                        stop=False,
                                        skip_group_check=True,
                                    )
                            nc.tensor.matmul(
                                pq,
                                wcc,
                                CST[0:H, j % 3, q * hf : (q + 1) * hf, :],
                                start=False,
                                stop=True,
                                skip_group_check=True,
                            )
                        nc.scalar.activation(
                            CST[0:H, (j + 1) % 3, 0:hf, :],
                            pc0[:, 0:256],
                            AF.Relu,
                            bias=bctx,
                        )
                        nc.vector.tensor_scalar(
                            CST[0:H, (j + 1) % 3, hf:, :],
                            pc1[:, 0:256],
                            bctx[:],
                            0.0,
                            op0=mybir.AluOpType.add,
                            op1=mybir.AluOpType.max,
                        )
                        if j < W_WARM:
                            nc.gpsimd.tensor_copy(
                                CAT[0 : H + 1, j + 1 : j + 2, :],
                                CST[:, (j + 1) % 3, 0:1, :],
                            )
                        else:
                            nc.gpsimd.tensor_copy(
                                CAT[0 : H + 1, j + 1 : j + 2 + 7 * L1 : L1, :],
                                CST[:, (j + 1) % 3, :, :],
                            )

                    for j in range(max(S_CH if stage >= 2 else 0, T // NCg)):
                        if j < T // NCg and j % 2 == 0:
                            h_pair(j)
                        if stage >= 2 and j < S_CH:
                            scan_step(j)

            # ---- G pass: GA[:, s, b] = M_hat @ [c_{s-1}; 1] for all b ----
            # (pools opened after scanp closed, reusing its SBUF columns)
            attn_stack = ExitStack()
            gap = attn_stack.enter_context(tc.tile_pool(name="gap", bufs=1))
            epool = attn_stack.enter_context(tc.tile_pool(name="epool", bufs=4))
            caepool = attn_stack.enter_context(tc.tile_pool(name="caepool", bufs=2))
            GA = gap.tile([H + 1, S + 1, BC], BF16)
            NB = (S + 1) * BC // 512  # 32 full matmuls; remainder below
            with tc.tile_pool(name="psG", bufs=4, space=bass.MemorySpace.PSUM) as psG:
                for k in range(NB if stage >= 3 else 0):
                    gp = psG.tile([H + 1, NCg, BC], F32, tag="gp")
                    nc.tensor.matmul(
                        gp,
                        mh_bf,
                        CAT[0 : H + 1, k * NCg : (k + 1) * NCg, :],
                        skip_group_check=True,
                    )
                    dst = GA[:, k * NCg : (k + 1) * NCg, :]
                    if k % 2 == 0:
                        nc.scalar.copy(dst, gp)
                    else:
                        nc.vector.tensor_copy(dst, gp)
                rem = (S + 1) - NB * NCg  # 2 blocks
                if stage >= 3:
                    gp = psG.tile([H + 1, NCg, BC], F32, tag="gp")
                    nc.tensor.matmul(
                        gp[:, 0:rem, :],
                        mh_bf,
                        CAT[0 : H + 1, NB * NCg : NB * NCg + rem, :],
                        skip_group_check=True,
                    )
                    nc.vector.tensor_copy(
                        GA[:, NB * NCg : NB * NCg + rem, :], gp[:, 0:rem, :]
                    )

            # ---- attention: pairs of batch elements ----
            with (
                tc.tile_pool(name="psS", bufs=2, space=bass.MemorySpace.PSUM) as psS,
                tc.tile_pool(name="psA", bufs=2, space=bass.MemorySpace.PSUM) as psA,
            ):
                for g in range(BC // 8 if stage >= 4 else 0):
                    do_aff = stage != 5
                    do_mm = stage != 6
                    acps = psA.tile([128, 2, 8, KP], F32, tag="ac")
                    caps = psA.tile([128, 8, 2, KP], F32, tag="ca")
                    for bi in range(8 if do_mm else 0):
                        b = g * 8 + bi
                        nc.tensor.matmul(
                            caps[:, bi, 0, :], CAT[0 : H + 1, 0:128, b], wae_bf
                        )
                        nc.tensor.matmul(
                            caps[:, bi, 1, :], CAT[0 : H + 1, 128:256, b], wae_bf
                        )
                    cae = caepool.tile([128, 8, 2, KP], BF16, tag="cae")
                    if do_mm:
                        nc.vector.tensor_copy(cae, caps)
                    for pi in range(4):
                        stp = psS.tile([128, 2, 512], F32, tag="st")
                        e = epool.tile([128, 2, 384], BF16, tag="e")
                        for kk in range(2):
                            b = g * 8 + pi * 2 + kk
                            # scores chunk s in [0,128): all t
                            nc.tensor.matmul(
                                stp[:, kk, 0:256],
                                CAT[0 : H + 1, 0:128, b],
                                GA[0 : H + 1, 1:S, b],
                            )
                            # scores chunk s in [128,256): t in [128,256)
                            nc.tensor.matmul(
                                stp[:, kk, 256:384],
                                CAT[0 : H + 1, 128:256, b],
                                GA[0 : H + 1, 129:S, b],
                            )
                        nc.scalar.activation(e, stp[:, :, 0:384], AF.Exp)
                        # causal mask: keep iff 1 - s_local + t_local >= 0
                        for kk in range(2 if do_aff else 0):
                            nc.gpsimd.affine_select(
                                e[:, kk, 0:256],
                                e[:, kk, 0:256],
                                pattern=[[1, 256]],
                                compare_op=mybir.AluOpType.is_ge,
                                fill=0.0,
                                base=1,
                                channel_multiplier=-1,
                            )
                            nc.gpsimd.affine_select(
                                e[:, kk, 256:384],
                                e[:, kk, 256:384],
                                pattern=[[1, 128]],
                                compare_op=mybir.AluOpType.is_ge,
                                fill=0.0,
                                base=1,
                                channel_multiplier=-1,
                            )
                        for kk in range(2 if do_mm else 0):
                            bi = pi * 2 + kk
                            nc.tensor.matmul(
                                acps[:, 0, bi, :],
                                e[:, kk, 0:128],
                                cae[:, bi, 0, :],
                            )
                            nc.tensor.matmul(
                                acps[:, 1, bi, :],
                                e[:, kk, 128:256],
                                cae[:, bi, 0, :],
                                start=True,
                                stop=False,
                            )
                            nc.tensor.matmul(
                                acps[:, 1, bi, :],
                                e[:, kk, 256:384],
                                cae[:, bi, 1, :],
                                start=False,
                                stop=True,
                            )
                    if do_mm:
                        nc.vector.tensor_copy(ACTS[:, :, g * 8 : (g + 1) * 8, :], acps)

            if stage < 4 or stage == 6:
                nc.vector.memset(ACTS[:], 0.0)
            nc.sync.dma_start(acts_raw[:], ACTS[:])
            nc.sync.dma_start(c_edge[:, 0:1, :], CAT[0:H, 128:129, :])
            nc.sync.dma_start(c_edge[:, 1:2, :], CAT[0:H, S - 1 : S, :])
            attn_stack.close()
        rep_stack.close()

    nc.compile()
    return nc


def _get_nc(reps=1, stage=4):
    key = ("nc", reps, stage)
    if key not in _CACHE:
        _CACHE[key] = _build_nc(reps, stage)
    return _CACHE[key]


def _prep_inputs(x, W_in, b_in, W_ctx, b_ctx, W_key, b_key, W_q, b_q,
                 first_context, W_act, b_act):
    x = np.asarray(x, np.float32)
    Wctx = np.asarray(W_ctx, np.float32)
    wpack = np.zeros((D, CW), np.float32)
    wpack[0:D, C_WP1 : C_WP1 + H] = np.asarray(W_in, np.float32).T
    wpack[0:H, C_WCC : C_WCC + H] = Wctx[:, 0:H].T
    wpack[0:H, C_WCH : C_WCH + H] = Wctx[:, H:].T
    Wk = np.asarray(W_key, np.float64)
    Wq = np.asarray(W_q, np.float64)
    bk = np.asarray(b_key, np.float64)
    bq = np.asarray(b_q, np.float64)
    mhm = np.zeros((H + 1, H + 1), np.float64)
    mhm[0:H, 0:H] = Wk.T @ Wq
    mhm[0:H, H] = Wk.T @ bq
    mhm[H, 0:H] = bk @ Wq
    mhm[H, H] = bk @ bq
    wpack[0 : H + 1, C_MH : C_MH + H + 1] = np.ascontiguousarray(mhm.T).astype(
        np.float32
    )
    w_ae = np.zeros((H + 1, KP), np.float32)
    w_ae[0:H, 0:A] = np.asarray(W_act, np.float32).T
    w_ae[H, A] = 1.0
    wpack[0 : H + 1, C_WAE : C_WAE + KP] = w_ae
    wpack[0:H, C_BIN] = np.asarray(b_in, np.float32)
    wpack[0:H, C_BCTX] = np.asarray(b_ctx, np.float32)
    wpack[0:H, C_C0 : C_C0 + NCg * BC] = np.asarray(first_context, np.float32)[
        :, None
    ]
    wpack[H, C_C0 : C_C0 + NCg * BC] = 1.0  # ones row rides along with commits

    perm = np.empty(T, np.int64)
    for j in range(L1):
        for i in range(NCg):
            perm[j * NCg + i] = i * L1 + j
    for j in range(L1, S_CH):
        perm[NCg * L1 + (j - L1)] = 7 * L1 + j
    in_maps = []
    for c in range(N_CORES):
        xc = x[:, c * BC : (c + 1) * BC, :]  # [T, BC, D]
        xTc = np.ascontiguousarray(xc.transpose(2, 0, 1)[:, perm, :])  # [D, T, BC]
        in_maps.append({"xT": xTc, "wpack": wpack})
    return in_maps


def _postprocess(results, W_key, b_key, W_q, b_q, W_act, b_act):
    W_key = np.asarray(W_key, np.float64)
    W_q = np.asarray(W_q, np.float64)
    W_act = np.asarray(W_act, np.float64)
    b_key = np.asarray(b_key, np.float64)
    b_q = np.asarray(b_q, np.float64)
    b_act = np.asarray(b_act, np.float32)
    out = np.empty((T, B, A), np.float32)
    for c in range(N_CORES):
        raw = np.asarray(results[c]["acts_raw"], np.float64)  # [128, 2, BC, KP]
        tmp = raw.transpose(1, 0, 2, 3).reshape(T, BC, KP)
        num = tmp[..., 0:A]
        den = tmp[..., A]
        ce = np.asarray(results[c]["c_edge"], np.float64)  # [H, 2, BC]
        # diagonal terms the device skips: at t, key_{t+1} comes from c_t
        for t_fix, idx in ((127, 0), (255, 1)):
            cv = ce[:, idx, :]  # [H, BC]
            key = W_key @ cv + b_key[:, None]
            q = W_q @ cv + b_q[:, None]
            e = np.exp((key * q).sum(0))  # [BC]
            num[t_fix] += e[:, None] * (cv.T @ W_act.T)
            den[t_fix] += e
        out[:, c * BC : (c + 1) * BC, :] = (num / den[..., None]).astype(
            np.float32
        ) + b_act
    return out


def _get_runner():
    if "runner" in _CACHE:
        return _CACHE["runner"]
    import jax
    from jax.experimental.shard_map import shard_map
    from jax.sharding import Mesh, PartitionSpec

    from concourse import bass2jax, mybir as mb

    nc = _get_nc()
    bass2jax.install_neuronx_cc_hook()
    assert nc.dbg_addr is None
    partition_name = nc.partition_id_tensor.name if nc.partition_id_tensor else None

    in_names, out_names, out_avals, zero_outs = [], [], [], []
    for alloc in nc.m.functions[0].allocations:
        if not isinstance(alloc, mb.MemoryLocationSet):
            continue
        name = alloc.memorylocations[0].name
        if alloc.kind == "ExternalInput":
            in_names.append(name)
        elif alloc.kind == "ExternalOutput":
            shape = tuple(alloc.tensor_shape)
            dtype = mb.dt.np(alloc.dtype)
            out_names.append(name)
            out_avals.append(jax.core.ShapedArray(shape, dtype))
            zero_outs.append(np.zeros(shape, dtype))
    if partition_name is not None:
        in_names = [n for n in in_names if n != partition_name]
    n_params = len(in_names)
    all_names = in_names + out_names
    if partition_name is not None:
        all_names = all_names + [partition_name]
    donate = tuple(range(n_params, n_params + len(out_names)))

    def _body(*args):
        operands = list(args)
        if partition_name is not None:
            operands.append(bass2jax.partition_id_tensor())
        outs = bass2jax._bass_exec_p.bind(
            *operands,
            out_avals=tuple(out_avals),
            in_names=tuple(all_names),
            out_names=tuple(out_names),
            lowering_input_output_aliases=(),
            sim_require_finite=True,
            sim_require_nnan=True,
            nc=nc,
        )
        return tuple(outs)

    devices = jax.devices()[:N_CORES]
    mesh = Mesh(np.asarray(devices), ("core",))
    specs = (PartitionSpec("core"),) * (n_params + len(out_names))
    sharded = jax.jit(
        shard_map(
            _body,
            mesh=mesh,
            in_specs=specs,
            out_specs=(PartitionSpec("core"),) * len(out_names),
            check_rep=False,
        ),
        donate_argnums=donate,
        keep_unused=True,
    )

    def run(in_maps):
        concat_in = [
            np.concatenate([in_maps[c][n] for c in range(N_CORES)], axis=0)
            for n in in_names
        ]
        concat_zero = [
            np.zeros((N_CORES * z.shape[0], *z.shape[1:]), z.dtype) for z in zero_outs
        ]
        out_arrs = sharded(*concat_in, *concat_zero)
        return [
            {
                n: np.asarray(out_arrs[i]).reshape(N_CORES, *out_avals[i].shape)[c]
                for i, n in enumerate(out_names)
            }
            for c in range(N_CORES)
        ]

    run.sharded = sharded
    run.in_names = in_names
    run.out_names = out_names
    run.out_avals = out_avals
    run.zero_outs = zero_outs
    _CACHE["runner"] = run
    return run


def kernel(**inputs):
    run = _get_runner()
    in_maps = _prep_inputs(**inputs)
    results = run(in_maps)
    return _postprocess(
        results,
        inputs["W_key"],
        inputs["b_key"],
        inputs["W_q"],
        inputs["b_q"],
        inputs["W_act"],
        inputs["b_act"],
    )


# revision 10
# speedup vs baseline: 1.2589x; 1.2589x over previous
"""AttentiveRNN Trainium2 kernel, v2.

Same math as the baseline kernel (8-way parallel warm-up scan + folded
attention), restructured to cut instruction count and device time:

- x DMA'd in 4 big slabs; h = relu(W_in x + b_in) hoisted into one batched
  pass (32 matmuls) before the scan instead of per-step matmuls.
- Scan critical path: per step only the wcc matmul sits between relu(j-1)
  and relu(j); the wch@h matmul is issued first (PSUM accumulate order is
  commutative) so PE does it while waiting.
- G = M_hat @ [C;1] batched over all 64 batch elements in one 33-matmul
  pass (replaces 64 per-b matmuls + 64 DVE copies).
- Attention processed in pairs: both score chunks of both elements of a
  pair land in one 2-bank PSUM tile, giving ONE exp over [128,2,384] and
  two paired affine_select masks per pair.
- All small weights packed into a single input tensor (2 inputs + 2
  outputs total vs 10+3).

Host postprocess (unchanged): softmax normalization, + b_act, and the two
diagonal terms the device skips ((t=127,s=128) and (t=255,s=256)).
"""

import sys
from contextlib import ExitStack

sys.path.insert(0, "/opt/trn_rl_repo")

import numpy as np

import concourse.bacc as bacc
import concourse.bass as bass
import concourse.tile as tile
from concourse import mybir

T, B, D, H, K, A = 256, 512, 128, 50, 5, 4
KP = 6  # K padded even so caps/acps matmuls stay on the f32r fast path
N_CORES = 8
BC = B // N_CORES  # 64 batch elements per core
S = T + 1  # context count
F32 = mybir.dt.float32
F32R = mybir.dt.float32r
BF16 = mybir.dt.bfloat16
AF = mybir.ActivationFunctionType

NCg = 8  # parallel scan chunks
W_WARM = 8  # warmup steps; relu recurrence contracts ~0.28x/step -> ~4e-5 rel
L1 = (T - W_WARM) // NCg  # 31
S_CH = L1 + W_WARM  # 39 scan steps

# wpack column layout (per-core packed weights, [128, CW] f32 bits)
C_WP1 = 0          # [0:128, 0:50]   W_in^T
C_WCC = 50         # [0:50, 50:100]  Wctx[:, :H]^T
C_WCH = 100        # [0:50, 100:150] Wctx[:, H:]^T
C_MH = 150         # [0:51, 150:201] M_hat^T
C_WAE = 202        # [0:51, 202:208] [W_act^T | e_A | 0]
C_BIN = 208        # [0:50, 208]     b_in
C_BCTX = 209       # [0:50, 209]     b_ctx
C_C0 = 210         # [0:51, 210:722] c0 broadcast to 8*64 (+ones row 50)
CW = 722

_CACHE = {}


def _build_nc(reps=1, stage=4):
    # stage: 1=h only, 2=+scan, 3=+G, 4=full
    # 5=full minus affine_selects, 6=full minus caps/acps, 7=exp->DVE copy
    nc = bacc.Bacc("TRN2", target_bir_lowering=False, debug=False)

    # x columns pre-permuted on host: position j*8+i holds t=i*L1+j (j<L1),
    # tail positions 8*L1.. hold t=7*L1+j (j>=L1).
    xT = nc.dram_tensor("xT", [D, T, BC], F32R, kind="ExternalInput")
    wpack = nc.dram_tensor("wpack", [D, CW], F32R, kind="ExternalInput")

    acts_raw = nc.dram_tensor("acts_raw", [128, 2, BC, KP], F32, kind="ExternalOutput")
    c_edge = nc.dram_tensor("c_edge", [H, 2, BC], BF16, kind="ExternalOutput")

    with tile.TileContext(nc) as tc:
        rep_stack = ExitStack()
        if reps > 1:
            rep_stack.enter_context(
                tc.For_i(
                    0,
                    reps,
                    1,
                    hint_engines=(mybir.EngineType.PE,),
                    staggered_reset=True,
                )
            )
        with tc.tile_pool(name="persist", bufs=1) as persist:
            wsb = persist.tile([D, CW], F32R, tag="wsb")
            nc.sync.dma_start(wsb, wpack[:])
            wp1 = wsb[0:D, C_WP1 : C_WP1 + H]
            wcc = wsb[0:H, C_WCC : C_WCC + H]
            wch = wsb[0:H, C_WCH : C_WCH + H]
            mh = wsb[0 : H + 1, C_MH : C_MH + H + 1]
            wae = wsb[0 : H + 1, C_WAE : C_WAE + KP]
            bin_ = wsb[0:H, C_BIN : C_BIN + 1].bitcast(F32)
            bctx = wsb[0:H, C_BCTX : C_BCTX + 1].bitcast(F32)
            c0rep = wsb[0 : H + 1, C_C0 : C_C0 + NCg * BC]

            mh_bf = persist.tile([H + 1, H + 1], BF16, tag="mh_bf")
            nc.vector.tensor_copy(mh_bf, mh.bitcast(F32))
            wae_bf = persist.tile([H + 1, KP], BF16, tag="wae_bf")
            nc.vector.tensor_copy(wae_bf, wae.bitcast(F32))

            # CAT rows 0-49: context c_{s-1} per block s; row 50: ones.
            # The ones row rides along with every commit: CST/c0rep carry a
            # constant-1.0 row 50 that each 51-row commit copies into CAT.
            CAT = persist.tile([H + 1, S + 1, BC], BF16)  # +1 pad block
            CST = persist.tile([H + 1, 3, NCg, BC], F32R)  # scan state (3-slot)
            ACTS = persist.tile([128, 2, BC, KP], F32)

            nc.gpsimd.tensor_copy(CAT[0 : H + 1, 0:1, :], c0rep[:, 0:BC])
            nc.gpsimd.tensor_copy(CAT[0 : H + 1, S : S + 1, :], c0rep[:, 0:BC])
            for sl3 in range(3):
                nc.gpsimd.tensor_copy(CST[:, sl3, :, :], c0rep[:])

            # ---- x DMA (8 slabs, double-buffered) interleaved with the
            # hoisted h pass AND the scan, so the scan starts immediately
            # (PE queue is in-order; h matmul j feeds scan step j).
            NSLAB = 8
            TB = T // NSLAB  # 32 blocks per slab
            HPS = TB // NCg  # 4 h-matmuls per slab
            with tc.tile_pool(name="scanp", bufs=1) as scanp:
                HA = scanp.tile([H, T, BC], F32R)  # h = relu(W_in x + b)
                with (
                    tc.tile_pool(name="xp", bufs=2) as xp,
                    tc.tile_pool(name="psH", bufs=2, space=bass.MemorySpace.PSUM) as psH,
                    tc.tile_pool(name="psC", bufs=2, space=bass.MemorySpace.PSUM) as psCp,
                ):
                    xbs = {}
                    xb0 = xp.tile([D, TB, BC], F32R, tag="xb")
                    xbs[0] = xb0
                    nc.sync.dma_start(xbs[0], xT[:, 0:TB, :])

                    def h_pair(k):
                        # h-matmuls k and k+1 share one PSUM tile + relu pair
                        sl, kk = divmod(k, HPS)
                        if kk == 0 and sl + 1 < NSLAB:
                            xbn = xp.tile([D, TB, BC], F32R, tag="xb")
                            xbs[sl + 1] = xbn
                            nc.sync.dma_start(
                                xbn, xT[:, (sl + 1) * TB : (sl + 2) * TB, :]
                            )
                        ph = psH.tile([H, 2 * NCg, BC], F32, tag="ph")
                        nc.tensor.matmul(
                            ph[:, 0:NCg, :],
                            wp1,
                            xbs[sl][:, kk * NCg : (kk + 1) * NCg, :],
                            skip_group_check=True,
                        )
                        nc.tensor.matmul(
                            ph[:, NCg : 2 * NCg, :],
                            wp1,
                            xbs[sl][:, (kk + 1) * NCg : (kk + 2) * NCg, :],
                            skip_group_check=True,
                        )
                        hs = NCg  # bank boundary of ph
                        dst = HA[:, k * NCg : (k + 2) * NCg, :]
                        nc.scalar.activation(
                            dst[:, 0:hs, :], ph[:, 0:hs, :], AF.Relu, bias=bin_
                        )
                        nc.vector.tensor_scalar(
                            dst[:, hs:, :],
                            ph[:, hs:, :],
                            bin_[:],
                            0.0,
                            op0=mybir.AluOpType.add,
                            op1=mybir.AluOpType.max,
                        )

                    def scan_step(j):
                        # pc: [H, 2, 512] = one PSUM bank per half, so the
                        # ACT and DVE relu halves run truly parallel (same-
                        # bank pairs are serialized by the bank tracker).
                        # Each wcc half reads/writes only its own chunk
                        # range, so the two chains decouple step-to-step.
                        pc0 = psCp.tile([H, 256], F32, tag="pc0")
                        pc1 = psCp.tile([H, 256], F32, tag="pc1")
                        pcs = (pc0, pc1)
                        hf = NCg // 2
                        for q in range(2):
                            pq = pcs[q][:, 0:256]
                            # wch@h first: no dep on relu(j-1).
                            if j < L1:
                                nc.tensor.matmul(
                                    pq,
                                    wch,
                                    HA[:, j * NCg + q * hf : j * NCg + (q + 1) * hf, :],
                                    start=True,
                                    stop=False,
                                    skip_group_check=True,
                                )
                            else:
                                jp = j - L1
                                if q == 0:
                                    nc.tensor.matmul(
                                        pq,
                                        wch,
                                        HA[:, jp * NCg + 1 : jp * NCg + 1 + hf, :],
                                        start=True,
                                        stop=False,
                                        skip_group_check=True,
                                    )
                                else:
                                    nc.tensor.matmul(
                                        pc1[:, 0:192],
                                        wch,
                                        HA[:, jp * NCg + 5 : jp * NCg + NCg, :],
                                        start=True,
                                        stop=False,
                                        skip_group_check=True,
                                    )
                                    p = NCg * L1 + jp
                                    nc.tensor.matmul(
                                        pc1[:, 192:256],
                                        wch,
                                        HA[:, p : p + 1, :],
                                        start=False,
                                        stop=False,
                                        skip_group_check=True,
                                    )
                            nc.tensor.matmul(
                                pq,
                                wcc,
                                CST[0:H, j % 3, q * hf : (q + 1) * hf, :],
                                start=False,
                                stop=True,
                                skip_group_check=True,
                            )
                        nc.scalar.activation(
                            CST[0:H, (j + 1) % 3, 0:hf, :],
                            pc0[:, 0:256],
                            AF.Relu,
                            bias=bctx,
                        )
                        nc.vector.tensor_scalar(
                            CST[0:H, (j + 1) % 3, hf:, :],
                            pc1[:, 0:256],
                            bctx[:],
                            0.0,
                            op0=mybir.AluOpType.add,
                            op1=mybir.AluOpType.max,
                        )
                        if j < W_WARM:
                            nc.gpsimd.tensor_copy(
                                CAT[0 : H + 1, j + 1 : j + 2, :],
                                CST[:, (j + 1) % 3, 0:1, :],
                            )
                        else:
                            nc.gpsimd.tensor_copy(
                                CAT[0 : H + 1, j + 1 : j + 2 + 7 * L1 : L1, :],
                                CST[:, (j + 1) % 3, :, :],
                            )

                    for j in range(max(S_CH if stage >= 2 else 0, T // NCg)):
                        if j < T // NCg and j % 2 == 0:
                            h_pair(j)
                        if stage >= 2 and j < S_CH:
                            scan_step(j)

            # ---- G pass: GA[:, s, b] = M_hat @ [c_{s-1}; 1] for all b ----
            # (pools opened after scanp closed, reusing its SBUF columns)
            attn_stack = ExitStack()
            gap = attn_stack.enter_context(tc.tile_pool(name="gap", bufs=1))
            epool = attn_stack.enter_context(tc.tile_pool(name="epool", bufs=4))
            caepool = attn_stack.enter_context(tc.tile_pool(name="caepool", bufs=2))
            GA = gap.tile([H + 1, S + 1, BC], BF16)
            NB = (S + 1) * BC // 512  # 32 full matmuls; remainder below
            with tc.tile_pool(name="psG", bufs=4, space=bass.MemorySpace.PSUM) as psG:
                for k in range(NB if stage >= 3 else 0):
                    gp = psG.tile([H + 1, NCg, BC], F32, tag="gp")
                    nc.tensor.matmul(
                        gp,
                        mh_bf,
                        CAT[0 : H + 1, k * NCg : (k + 1) * NCg, :],
                        skip_group_check=True,
                    )
                    dst = GA[:, k * NCg : (k + 1) * NCg, :]
                    if k % 2 == 0:
                        nc.scalar.copy(dst, gp)
                    else:
                        nc.vector.tensor_copy(dst, gp)
                rem = (S + 1) - NB * NCg  # 2 blocks
                if stage >= 3:
                    gp = psG.tile([H + 1, NCg, BC], F32, tag="gp")
                    nc.tensor.matmul(
                        gp[:, 0:rem, :],
                        mh_bf,
                        CAT[0 : H + 1, NB * NCg : NB * NCg + rem, :],
                        skip_group_check=True,
                    )
                    nc.vector.tensor_copy(
                        GA[:, NB * NCg : NB * NCg + rem, :], gp[:, 0:rem, :]
                    )

            # ---- attention: pairs of batch elements ----
            with (
                tc.tile_pool(name="psS", bufs=2, space=bass.MemorySpace.PSUM) as psS,
                tc.tile_pool(name="psA", bufs=2, space=bass.MemorySpace.PSUM) as psA,
            ):
                for g in range(BC // 8 if stage >= 4 else 0):
                    do_aff = stage != 5
                    do_mm = stage != 6
                    acps = psA.tile([128, 2, 8, KP], F32, tag="ac")
                    caps = psA.tile([128, 8, 2, KP], F32, tag="ca")
                    for bi in range(8 if do_mm else 0):
                        b = g * 8 + bi
                        nc.tensor.matmul(
                            caps[:, bi, 0, :], CAT[0 : H + 1, 0:128, b], wae_bf
                        )
                        nc.tensor.matmul(
                            caps[:, bi, 1, :], CAT[0 : H + 1, 128:256, b], wae_bf
                        )
                    cae = caepool.tile([128, 8, 2, KP], BF16, tag="cae")
                    if do_mm:
                        nc.vector.tensor_copy(cae, caps)
                    for pi in range(4):
                        stp = psS.tile([128, 2, 512], F32, tag="st")
                        e = epool.tile([128, 2, 384], BF16, tag="e")
                        for kk in range(2):
                            b = g * 8 + pi * 2 + kk
                            # scores chunk s in [0,128): all t
                            nc.tensor.matmul(
                                stp[:, kk, 0:256],
                                CAT[0 : H + 1, 0:128, b],
                                GA[0 : H + 1, 1:S, b],
                            )
                            # scores chunk s in [128,256): t in [128,256)
                            nc.tensor.matmul(
                                stp[:, kk, 256:384],
                                CAT[0 : H + 1, 128:256, b],
                                GA[0 : H + 1, 129:S, b],
                            )
                        for kk in range(2):
                            nc.scalar.activation(
                                e[:, kk, :], stp[:, kk, 0:384], AF.Exp
                            )
                        # causal mask: keep iff 1 - s_local + t_local >= 0
                        for kk in range(2 if do_aff else 0):
                            nc.gpsimd.affine_select(
                                e[:, kk, 0:256],
                                e[:, kk, 0:256],
                                pattern=[[1, 256]],
                                compare_op=mybir.AluOpType.is_ge,
                                fill=0.0,
                                base=1,
                                channel_multiplier=-1,
                            )
                            nc.gpsimd.affine_select(
                                e[:, kk, 256:384],
                                e[:, kk, 256:384],
                                pattern=[[1, 128]],
                                compare_op=mybir.AluOpType.is_ge,
                                fill=0.0,
                                base=1,
                                channel_multiplier=-1,
                            )
                        for kk in range(2 if do_mm else 0):
                            bi = pi * 2 + kk
                            nc.tensor.matmul(
                                acps[:, 0, bi, :],
                                e[:, kk, 0:128],
                                cae[:, bi, 0, :],
                            )
                            nc.tensor.matmul(
                                acps[:, 1, bi, :],
                                e[:, kk, 128:256],
                                cae[:, bi, 0, :],
                                start=True,
                                stop=False,
                            )
                            nc.tensor.matmul(
                                acps[:, 1, bi, :],
                                e[:, kk, 256:384],
                                cae[:, bi, 1, :],
                                start=False,
                                stop=True,
                            )
                    if do_mm:
                        nc.vector.tensor_copy(ACTS[:, :, g * 8 : (g + 1) * 8, :], acps)

            if stage < 4 or stage == 6:
                nc.vector.memset(ACTS[:], 0.0)
            nc.sync.dma_start(acts_raw[:], ACTS[:])
            nc.sync.dma_start(c_edge[:, 0:1, :], CAT[0:H, 128:129, :])
            nc.sync.dma_start(c_edge[:, 1:2, :], CAT[0:H, S - 1 : S, :])
            attn_stack.close()
        rep_stack.close()

    nc.compile()
    return nc


def _get_nc(reps=1, stage=4):
    key = ("nc", reps, stage)
    if key not in _CACHE:
        _CACHE[key] = _build_nc(reps, stage)
    return _CACHE[key]


def _prep_inputs(x, W_in, b_in, W_ctx, b_ctx, W_key, b_key, W_q, b_q,
                 first_context, W_act, b_act):
    x = np.asarray(x, np.float32)
    Wctx = np.asarray(W_ctx, np.float32)
    wpack = np.zeros((D, CW), np.float32)
    wpack[0:D, C_WP1 : C_WP1 + H] = np.asarray(W_in, np.float32).T
    wpack[0:H, C_WCC : C_WCC + H] = Wctx[:, 0:H].T
    wpack[0:H, C_WCH : C_WCH + H] = Wctx[:, H:].T
    Wk = np.asarray(W_key, np.float64)
    Wq = np.asarray(W_q, np.float64)
    bk = np.asarray(b_key, np.float64)
    bq = np.asarray(b_q, np.float64)
    mhm = np.zeros((H + 1, H + 1), np.float64)
    mhm[0:H, 0:H] = Wk.T @ Wq
    mhm[0:H, H] = Wk.T @ bq
    mhm[H, 0:H] = bk @ Wq
    mhm[H, H] = bk @ bq
    wpack[0 : H + 1, C_MH : C_MH + H + 1] = np.ascontiguousarray(mhm.T).astype(
        np.float32
    )
    w_ae = np.zeros((H + 1, KP), np.float32)
    w_ae[0:H, 0:A] = np.asarray(W_act, np.float32).T
    w_ae[H, A] = 1.0
    wpack[0 : H + 1, C_WAE : C_WAE + KP] = w_ae
    wpack[0:H, C_BIN] = np.asarray(b_in, np.float32)
    wpack[0:H, C_BCTX] = np.asarray(b_ctx, np.float32)
    wpack[0:H, C_C0 : C_C0 + NCg * BC] = np.asarray(first_context, np.float32)[
        :, None
    ]
    wpack[H, C_C0 : C_C0 + NCg * BC] = 1.0  # ones row rides along with commits

    perm = np.empty(T, np.int64)
    for j in range(L1):
        for i in range(NCg):
            perm[j * NCg + i] = i * L1 + j
    for j in range(L1, S_CH):
        perm[NCg * L1 + (j - L1)] = 7 * L1 + j
    in_maps = []
    for c in range(N_CORES):
        xc = x[:, c * BC : (c + 1) * BC, :]  # [T, BC, D]
        xTc = np.ascontiguousarray(xc.transpose(2, 0, 1)[:, perm, :])  # [D, T, BC]
        in_maps.append({"xT": xTc, "wpack": wpack})
    return in_maps


def _postprocess(results, W_key, b_key, W_q, b_q, W_act, b_act):
    W_key = np.asarray(W_key, np.float64)
    W_q = np.asarray(W_q, np.float64)
    W_act = np.asarray(W_act, np.float64)
    b_key = np.asarray(b_key, np.float64)
    b_q = np.asarray(b_q, np.float64)
    b_act = np.asarray(b_act, np.float32)
    out = np.empty((T, B, A), np.float32)
    for c in range(N_CORES):
        raw = np.asarray(results[c]["acts_raw"], np.float64)  # [128, 2, BC, KP]
        tmp = raw.transpose(1, 0, 2, 3).reshape(T, BC, KP)
        num = tmp[..., 0:A]
        den = tmp[..., A]
        ce = np.asarray(results[c]["c_edge"], np.float64)  # [H, 2, BC]
        # diagonal terms the device skips: at t, key_{t+1} comes from c_t
        for t_fix, idx in ((127, 0), (255, 1)):
            cv = ce[:, idx, :]  # [H, BC]
            key = W_key @ cv + b_key[:, None]
            q = W_q @ cv + b_q[:, None]
            e = np.exp((key * q).sum(0))  # [BC]
            num[t_fix] += e[:, None] * (cv.T @ W_act.T)
            den[t_fix] += e
        out[:, c * BC : (c + 1) * BC, :] = (num / den[..., None]).astype(
            np.float32
        ) + b_act
    return out


def _get_runner():
    if "runner" in _CACHE:
        return _CACHE["runner"]
    import jax
    from jax.experimental.shard_map import shard_map
    from jax.sharding import Mesh, PartitionSpec

    from concourse import bass2jax, mybir as mb

    nc = _get_nc()
    bass2jax.install_neuronx_cc_hook()
    assert nc.dbg_addr is None
    partition_name = nc.partition_id_tensor.name if nc.partition_id_tensor else None

    in_names, out_names, out_avals, zero_outs = [], [], [], []
    for alloc in nc.m.functions[0].allocations:
        if not isinstance(alloc, mb.MemoryLocationSet):
            continue
        name = alloc.memorylocations[0].name
        if alloc.kind == "ExternalInput":
            in_names.append(name)
        elif alloc.kind == "ExternalOutput":
            shape = tuple(alloc.tensor_shape)
            dtype = mb.dt.np(alloc.dtype)
            out_names.append(name)
            out_avals.append(jax.core.ShapedArray(shape, dtype))
            zero_outs.append(np.zeros(shape, dtype))
    if partition_name is not None:
        in_names = [n for n in in_names if n != partition_name]
    n_params = len(in_names)
    all_names = in_names + out_names
    if partition_name is not None:
        all_names = all_names + [partition_name]
    donate = tuple(range(n_params, n_params + len(out_names)))

    def _body(*args):
        operands = list(args)
        if partition_name is not None:
            operands.append(bass2jax.partition_id_tensor())
        outs = bass2jax._bass_exec_p.bind(
            *operands,
            out_avals=tuple(out_avals),
            in_names=tuple(all_names),
            out_names=tuple(out_names),
            lowering_input_output_aliases=(),
            sim_require_finite=True,
            sim_require_nnan=True,
            nc=nc,
        )
        return tuple(outs)

    devices = jax.devices()[:N_CORES]
    mesh = Mesh(np.asarray(devices), ("core",))
    specs = (PartitionSpec("core"),) * (n_params + len(out_names))
    sharded = jax.jit(
        shard_map(
            _body,
            mesh=mesh,
            in_specs=specs,
            out_specs=(PartitionSpec("core"),) * len(out_names),
            check_rep=False,
        ),
        donate_argnums=donate,
        keep_unused=True,
    )

    def run(in_maps):
        concat_in = [
            np.concatenate([in_maps[c][n] for c in range(N_CORES)], axis=0)
            for n in in_names
        ]
        concat_zero = [
            np.zeros((N_CORES * z.shape[0], *z.shape[1:]), z.dtype) for z in zero_outs
        ]
        out_arrs = sharded(*concat_in, *concat_zero)
        return [
            {
                n: np.asarray(out_arrs[i]).reshape(N_CORES, *out_avals[i].shape)[c]
                for i, n in enumerate(out_names)
            }
            for c in range(N_CORES)
        ]

    run.sharded = sharded
    run.in_names = in_names
    run.out_names = out_names
    run.out_avals = out_avals
    run.zero_outs = zero_outs
    _CACHE["runner"] = run
    return run


def kernel(**inputs):
    run = _get_runner()
    in_maps = _prep_inputs(**inputs)
    results = run(in_maps)
    return _postprocess(
        results,
        inputs["W_key"],
        inputs["b_key"],
        inputs["W_q"],
        inputs["b_q"],
        inputs["W_act"],
        inputs["b_act"],
    )
